# revision 38
# baseline (speedup 1.0000x reference)
"""Causal self-attention (BS=4, SL=2048, NE=1024, NH=16) on 8 trn2 NeuronCores.

Sharding (uniform SPMD program on all 8 cores):
  core c -> batch b = c//2, head-group g = c%2 (8 of 16 heads, 512 feats).
  Each core: QKV proj for its heads (full 2048 rows of its batch),
  causal attention for its 8 heads, then pairwise AllGather of y
  (cores 2b/2b+1), and out-proj for a 512-column half of the output.
  Host reassembles [4, 2048, 1024] from the 8 [2048, 512] halves.

v2 optimizations over the first working version:
  - x is pre-transposed on the HOST, so every x^T tile is a plain row
    DMA (the DMA-transpose ring was 50%-busy serialized before).
  - K bias dropped (softmax is invariant to a per-query additive shift)
    and V bias folded into the out-proj bias on the host (y = sum w_k
    (v_k + bv) = sum w_k v_k + bv since sum w_k = 1).
  - softmax 1/denom via reciprocal_approx_fast (5x faster DVE op;
    the exact [1,512] reciprocal was 3.35us x32 = 107us of DVE busy).
  - out-proj bias via a precomputed broadcast tile + tensor_add epilogue
    instead of a [1,128]x[128,512] PE matmul per block.
  - software-pipelined attention inner loop (S-matmul pairs run two
    ahead of PV pairs, so PV never head-of-line-blocks the PE queue
    behind the Exp activation), with projection/out-proj micro-ops
    (2 matmuls each) interleaved between slots to fill PE gaps.
  - out-projection of chunk i is interleaved into attention of panel
    i+2 instead of running serialized at the end.
  - fp16 output (cast back to fp32 on host).

v3 optimizations:
  - head-pair lockstep S matmuls: the two heads of a feat-group sit at
    PE row-tile positions (0,*) and (64,*) (K=64); adjacent matmuls at
    different row tiles execute CONCURRENTLY (measured 141ns vs 215ns
    per 512-wide matmul on HW).
  - one full gather per non-final panel (collectives are fixed-overhead
    dominated, ~4-10us each regardless of size; per-head-pair quarter
    gathers measured 2.4x slower per op and stalled both cores).
  - final panel gathers split 4/2/2 heads; out-proj of the final panel
    is two-phase: 6 of 8 contraction chunks run right after the last
    attention slot (covering the last norm + collective + readback
    latency), so only 8 matmuls wait on the final 128KB gather.
  - startup: panel-0 projection runs all-Q then all-K so the K matmuls
    hide the WK weight DMAs instead of stalling on them.
  NOTE: the final round is ACT(Exp)-bound (~50us of Exp vs ~27us of PE
  work); further S-matmul speedups there don't help wall clock.

Matmul operands in fp16 (full PE rate, fp32 PSUM accumulate).
Attention computed in S^T = K @ Q^T layout so that:
  - PV needs no transposes: Y^T[65,q] += [V|1]^T @ expS^T (row 64 = denom)
  - softmax normalization via gpsimd partition_broadcast of 1/denom
"""

import sys

if "/opt/trn_rl_repo" not in sys.path:
    sys.path.insert(0, "/opt/trn_rl_repo")

import numpy as np

import concourse.bass as bass
import concourse.mybir as mybir
import concourse.tile as tile
from concourse import bacc
from concourse.bass_utils import run_bass_kernel_spmd

F32 = mybir.dt.float32
F16 = mybir.dt.float16

# problem dims (hardcoded per spec)
BS, SL, NE, NH = 4, 2048, 1024, 16
HD = 64
N_CORES = 8


def build_nc(sl=SL, ne=NE, nh=NH, fake_collective=False):
    """Build the per-core Bass program. All 8 cores run this identically."""
    H = nh // 2          # local heads per core
    F = H * HD           # local feats (q/k/v width per core)
    FG = F // 128        # feat groups of 128 (2 heads each)
    CH = ne // 128       # contraction chunks for the projections
    PANEL = 512          # q-panel width
    NP = sl // PANEL     # number of q panels
    NKB = sl // 128      # number of 128-row k blocks
    OUTW = ne // 2       # out-proj columns computed per core
    VW = H * 65          # V' width (65-stride per head: 64 V cols + ones)
    N_CC = NP            # collective chunk == q panel
    SPAN = PANEL

    nc = bacc.Bacc("TRN2", target_bir_lowering=False, num_devices=N_CORES)

    xt = nc.dram_tensor("xt", [ne, sl], F16, kind="ExternalInput")
    wq = nc.dram_tensor("wq", [ne, F], F16, kind="ExternalInput")
    wk = nc.dram_tensor("wk", [ne, F], F16, kind="ExternalInput")
    wv = nc.dram_tensor("wv", [ne, F], F16, kind="ExternalInput")
    bq = nc.dram_tensor("bq", [F], F32, kind="ExternalInput")
    wo = nc.dram_tensor("wo", [ne, OUTW], F16, kind="ExternalInput")
    bo = nc.dram_tensor("bo", [OUTW], F32, kind="ExternalInput")
    out = nc.dram_tensor("out", [sl, OUTW], F16, kind="ExternalOutput")

    tri_dram = nc.inline_tensor(
        np.triu(np.ones((128, 128), dtype=np.float16)), name="tri_c")

    with tile.TileContext(nc) as tc:
        with (
            tc.tile_pool(name="consts", bufs=1) as consts,
            tc.tile_pool(name="xt", bufs=3) as xtp,
            tc.tile_pool(name="qt", bufs=2) as qtp,
            tc.tile_pool(name="persist", bufs=1) as persist,
            tc.tile_pool(name="es", bufs=8) as esp,
            tc.tile_pool(name="ny", bufs=2) as nyp,
            tc.tile_pool(name="misc", bufs=2) as misc,
            tc.tile_pool(name="psum", bufs=1, space="PSUM") as psp,
            tc.tile_pool(name="dram", bufs=1, space="DRAM") as dram,
        ):
            # ---- panel-0 x^T tiles + first weights so proj starts ASAP ----
            xT0 = [xtp.tile([128, PANEL], F16, tag=f"xt{c}", name=f"xT0_{c}")
                   for c in range(CH)]
            WQ = [persist.tile([128, F], F16, tag=f"wq{c}", name=f"WQ{c}")
                  for c in range(CH)]
            WK = [persist.tile([128, F], F16, tag=f"wk{c}", name=f"WK{c}")
                  for c in range(CH)]
            WV = [persist.tile([128, F], F16, tag=f"wv{c}", name=f"WV{c}")
                  for c in range(CH)]
            WO = [persist.tile([128, OUTW], F16, tag=f"wo{c}", name=f"WO{c}")
                  for c in range(CH)]
            for c in (0, 1):
                nc.sync.dma_start(out=xT0[c],
                                  in_=xt[c * 128:(c + 1) * 128, 0:PANEL])
                nc.sync.dma_start(out=WQ[c], in_=wq[c * 128:(c + 1) * 128, :])
            bqt = consts.tile([128, FG], F32)
            nc.sync.dma_start(out=bqt, in_=bq.rearrange("(g p) -> p g", p=128))
            for c in range(2, CH):
                nc.sync.dma_start(out=xT0[c],
                                  in_=xt[c * 128:(c + 1) * 128, 0:PANEL])
                nc.sync.dma_start(out=WQ[c], in_=wq[c * 128:(c + 1) * 128, :])
            for c in range(CH):
                sl_c = slice(c * 128, (c + 1) * 128)
                nc.sync.dma_start(out=WK[c], in_=wk[sl_c, :])
                nc.sync.dma_start(out=WV[c], in_=wv[sl_c, :])

            # ---- constants ----
            tri = consts.tile([128, 128], F16)
            nc.sync.dma_start(out=tri, in_=tri_dram[:])
            bo_row = consts.tile([1, OUTW], F32)
            nc.sync.dma_start(out=bo_row,
                              in_=bo.rearrange("(a n) -> a n", a=1))
            bo_bc = consts.tile([128, OUTW], F32)
            nc.gpsimd.partition_broadcast(bo_bc, bo_row)

            for c in range(CH):
                nc.sync.dma_start(out=WO[c], in_=wo[c * 128:(c + 1) * 128, :])

            # ---- persistent attention operands ----
            KT = [persist.tile([128, sl], F16, tag=f"kt{f}", name=f"KT{f}")
                  for f in range(FG)]
            VP = [persist.tile([128, VW], F16, tag=f"vp{k}", name=f"VP{k}")
                  for k in range(NKB)]

            y_local = dram.tile([N_CC, F, SPAN], F16)
            # AllGather split into head-halves: half A (heads 0-3) fires
            # mid-panel so only half B's 256KB transfer sits on the tail.
            # (Per-head-pair quarter collectives were tried and are ~2.4x
            # slower per op: the CC stream is fixed-overhead dominated and
            # 16 lockstep points per pair stall both cores.)
            # one full gather per non-final panel (collectives are
            # fixed-overhead dominated: fewer, bigger ops win; these are
            # consumed two rounds later so latency is hidden)
            y_allN = dram.tile([N_CC - 1, 2, F, SPAN], F16)
            # final panel: heads 0-3 mid-round, then 4-5 / 6-7 so only a
            # 128KB gather sits on the tail
            y_allA = dram.tile([2, F // 2, SPAN], F16)
            y_allF1 = dram.tile([2, 128, SPAN], F16)
            y_allF2 = dram.tile([2, 128, SPAN], F16)

            QTs = {}
            y_rows = {}

            def emit_xT(p):
                tiles = [xtp.tile([128, PANEL], F16, tag=f"xt{c}",
                                  name=f"xT{c}") for c in range(CH)]
                for c in range(CH):
                    nc.sync.dma_start(
                        out=tiles[c],
                        in_=xt[c * 128:(c + 1) * 128,
                               p * PANEL:(p + 1) * PANEL])
                return tiles

            # ---------- projection micro-ops (2 matmuls each) ----------
            def proj_micros(p, xT):
                QT = [qtp.tile([128, PANEL], F16, tag=f"qt{f}",
                               name=f"QT{f}") for f in range(FG)]
                QTs[p] = QT
                micros = []

                def qk_mms(f, wtiles, st, c0):
                    def go():
                        if 'ps' not in st:
                            st['ps'] = psp.tile([128, PANEL], F32, tag="acc",
                                                bufs=2, name="ps_a")
                        for c in (c0, c0 + 1):
                            nc.tensor.matmul(
                                st['ps'], wtiles[c][:, f * 128:(f + 1) * 128],
                                xT[c], start=(c == 0), stop=(c == CH - 1))
                    return go

                def q_epi(f, st):
                    def go():
                        nc.vector.tensor_scalar_add(
                            QT[f], st['ps'], bqt[:, f:f + 1])
                    return go

                def k_epi(f, st):
                    def go():
                        nc.vector.tensor_copy(
                            KT[f][:, p * PANEL:(p + 1) * PANEL], st['ps'])
                    return go

                def v_mms(sub, st, c0):
                    def go():
                        if 'ps' not in st:
                            st['ps'] = psp.tile([128, F], F32, tag="acc",
                                                bufs=2, name="ps_v")
                        for c in (c0, c0 + 1):
                            nc.tensor.matmul(
                                st['ps'], xT[c][:, sub * 128:(sub + 1) * 128],
                                WV[c], start=(c == 0), stop=(c == CH - 1))
                    return go

                def v_epi(sub, st):
                    def go():
                        kb = p * 4 + sub
                        vp3 = VP[kb].rearrange("p (h e) -> p h e", e=65)
                        nc.vector.memset(vp3[:, :, 64:65], 1.0)
                        nc.vector.tensor_copy(
                            vp3[:, :, 0:64],
                            st['ps'].rearrange("p (h d) -> p h d", d=64))
                    return go

                if p == 0:
                    # startup: all-Q first, then all-K — the K matmuls run
                    # ~7us of Q work after WQ lands, hiding the WK DMAs
                    # instead of stalling on them per f-group
                    for f in range(FG):
                        stq = {}
                        for c0 in range(0, CH, 2):
                            micros.append(qk_mms(f, WQ, stq, c0))
                        micros.append(q_epi(f, stq))
                    for f in range(FG):
                        stk = {}
                        for c0 in range(0, CH, 2):
                            micros.append(qk_mms(f, WK, stk, c0))
                        micros.append(k_epi(f, stk))
                else:
                    for f in range(FG):
                        stq, stk = {}, {}
                        for c0 in range(0, CH, 2):
                            micros.append(qk_mms(f, WQ, stq, c0))
                        micros.append(q_epi(f, stq))
                        for c0 in range(0, CH, 2):
                            micros.append(qk_mms(f, WK, stk, c0))
                        micros.append(k_epi(f, stk))
                for sub in range(4):
                    stv = {}
                    for c0 in range(0, CH, 2):
                        micros.append(v_mms(sub, stv, c0))
                    micros.append(v_epi(sub, stv))
                return micros

            # ---------- attention slots (software-pipelined) ----------
            # The two heads of a feat-group run in lockstep: their S
            # matmuls have K=64 and sit at PE row-tile positions (0,*) and
            # (64,*), which the PE executes CONCURRENTLY when adjacent
            # (measured 1.5x: 141ns vs 215ns per 512-wide matmul).

            # Cross-round Exp rebalance: the last round's attention has
            # ~50us of Exp vs ~48us of PE work while earlier rounds have
            # ACT slack.  S+exp of a panel's early off-diagonal key groups
            # runs one round EARLY (Q(p) is projected mid-round p; K for
            # those blocks already exists), parking es in SBUF; the next
            # round only runs their PV.
            pre_es = {}

            def pre_spair(pp, f, j, nbufs):
                def go():
                    QT = QTs[pp]
                    pss, ess = [], []
                    for hh in (0, 1):
                        pss.append(psp.tile([128, 2 * PANEL], F32,
                                            tag="s", bufs=2, name="ps_s"))
                        # per-panel tag sized to the exact allocation
                        # count: every pre-es gets a fresh slot, so no
                        # buffer-rotation wait can cycle with the PV reads
                        # that release earlier panels' slots
                        ess.append(esp.tile([128, 2 * PANEL], F16,
                                            tag=f"pre_es{pp}", bufs=nbufs,
                                            name="pre_es"))
                    for jj in (0, 1):
                        kb = 2 * j + jj
                        for hh in (0, 1):
                            row = hh * 64
                            nc.tensor.matmul(
                                pss[hh][:, jj * PANEL:(jj + 1) * PANEL],
                                KT[f][row:row + 64,
                                      kb * 128:(kb + 1) * 128],
                                QT[f][row:row + 64, 0:PANEL])
                    for hh in (0, 1):
                        nc.scalar.activation(
                            ess[hh], pss[hh],
                            mybir.ActivationFunctionType.Exp)
                    pre_es[(pp, f, j)] = ess
                return go

            def pair_slots(p, f):
                QT = QTs[p]
                nkb_p = 4 * p + 4
                npairs = nkb_p // 2
                sts = [{}, {}]

                def spair2(j):
                    pre = pre_es.pop((p, f, j), None)
                    if pre is not None:
                        def use_pre():
                            for hh in (0, 1):
                                sts[hh][('es', j)] = pre[hh]
                                sts[hh][('offs', j)] = [
                                    (2 * j + jj, 0, PANEL, jj * PANEL)
                                    for jj in (0, 1)]
                        return use_pre

                    def go():
                        diag = 2 * j >= 4 * p
                        pss = []
                        for hh in (0, 1):
                            st = sts[hh]
                            ps_s = psp.tile([128, 2 * PANEL], F32, tag="s",
                                            bufs=2, name="ps_s")
                            pss.append(ps_s)
                            es = esp.tile([128, 2 * PANEL], F16, tag="es",
                                          name="es")
                            offs = []
                            for jj in (0, 1):
                                kb = 2 * j + jj
                                d = max(0, (kb - 4 * p) * 128)
                                offs.append((kb, d, PANEL - d, jj * PANEL))
                            st[('es', j)] = es
                            st[('offs', j)] = offs
                        # interleave the two heads' matmuls so adjacent
                        # instructions target different PE row tiles
                        # (concurrent row-tile execution, measured 1.5x)
                        for jj in (0, 1):
                            for hh in (0, 1):
                                row = hh * 64
                                kb, d, n, o = sts[hh][('offs', j)][jj]
                                nc.tensor.matmul(
                                    pss[hh][:, o:o + n],
                                    KT[f][row:row + 64,
                                          kb * 128:(kb + 1) * 128],
                                    QT[f][row:row + 64, d:PANEL])
                        for hh in (0, 1):
                            st = sts[hh]
                            es = st[('es', j)]
                            # one full-width Exp per head even on diagonal
                            # groups: ACT fixed overhead outweighs the
                            # extra columns; unwritten (stale) columns are
                            # bounded S values that PV never reads
                            nc.scalar.activation(
                                es, pss[hh],
                                mybir.ActivationFunctionType.Exp)
                            if diag:
                                for kb, d, n, o in st[('offs', j)]:
                                    nc.vector.tensor_mul(
                                        es[:, o:o + 128], es[:, o:o + 128],
                                        tri)
                    return go

                def pvpair(hh, j):
                    def go():
                        st = sts[hh]
                        h = 2 * f + hh
                        if 'ps_y' not in st:
                            st['ps_y'] = psp.tile([65, PANEL], F32, tag="y",
                                                  bufs=2, name="ps_y")
                        ps_y = st['ps_y']
                        es = st.pop(('es', j))
                        for kb, d, n, o in st.pop(('offs', j)):
                            nc.tensor.matmul(
                                ps_y[:, d:PANEL],
                                VP[kb][:, h * 65:h * 65 + 65],
                                es[:, o:o + n],
                                start=(kb == 0), stop=(kb == nkb_p - 1))
                    return go

                def norm(hh):
                    def go():
                        st = sts[hh]
                        h = 2 * f + hh
                        ps_y = st['ps_y']
                        den_s = misc.tile([1, PANEL], F32, tag="den",
                                          name="den")
                        # on DVE, not ACT: an ACT-side copy queues ahead of
                        # the next head's Exps and stalls its PV pairs
                        nc.vector.tensor_copy(den_s, ps_y[64:65, :])
                        recip = misc.tile([1, PANEL], F32, tag="recip",
                                          name="recip")
                        nc.vector.reciprocal_approx_fast(out=recip,
                                                         in_=den_s)
                        bc = misc.tile([64, PANEL], F32, tag="bc", bufs=3,
                                       name="bc")
                        nc.gpsimd.partition_broadcast(bc, recip)
                        # shared tag: nY only stages norm -> y_local DMA,
                        # so 4 in-flight slots cover all heads
                        nY = nyp.tile([64, PANEL], F16, tag="ny", bufs=4,
                                      name="nY")
                        nc.vector.tensor_mul(nY, ps_y[0:64, :], bc)
                        nc.sync.dma_start(
                            out=y_local[p, h * 64:(h + 1) * 64, :], in_=nY)
                    return go

                slots = []
                for j in range(npairs):
                    slots.append(spair2(j))
                    if j >= 2:
                        slots.append(pvpair(0, j - 2))
                        slots.append(pvpair(1, j - 2))
                for j in (npairs - 2, npairs - 1):
                    slots.append(pvpair(0, j))
                    slots.append(pvpair(1, j))
                slots.append(norm(0))
                slots.append(norm(1))
                return slots

            # ---------- collective + out-proj ----------
            def emit_cc_heads(i, hs, he, y_out):
                """AllGather heads [hs, he) of panel i into y_out
                ([2, (he-hs)*64, SPAN]) and read the rows back.  Carries
                out-proj contraction chunks hs//2..he//2-1 (own) and
                4+hs//2.. (partner)."""
                nch = (he - hs) // 2
                y_in = y_local[i][hs * 64:he * 64, :]
                if fake_collective:
                    nc.sync.dma_start(out=y_out[0], in_=y_in)
                    nc.sync.dma_start(out=y_out[1], in_=y_in)
                else:
                    nc.gpsimd.collective_compute(
                        "AllGather",
                        mybir.AluOpType.bypass,
                        replica_groups=[[0, 1], [2, 3], [4, 5], [6, 7]],
                        ins=[y_in.opt()],
                        outs=[y_out.opt()],
                    )
                # one DMA for all c-chunks: t[p, (g a), s] <- y_out[g, a*128+p, s]
                # final-panel tags (partial head ranges) are used once ->
                # single buffer
                t = misc.tile([128, 2 * nch, PANEL], F16,
                              tag=f"yr{hs}_{he}",
                              bufs=2 if (hs, he) == (0, 8) else 1,
                              name=f"yr{hs}_{he}")
                nc.sync.dma_start(
                    out=t,
                    in_=y_out.rearrange("g (a p) s -> p (g a) s", p=128))
                cm = y_rows.setdefault(i, {})
                for a in range(nch):
                    cm[hs // 2 + a] = (t, a)
                    cm[4 + hs // 2 + a] = (t, nch + a)

            def op_mms(i, sb4, st, pair, first, last):
                def go():
                    if 'ps' not in st:
                        st['ps'] = psp.tile([128, OUTW], F32, tag="acc",
                                            bufs=2, name="ps_o")
                    y_row = y_rows[i]
                    for k, c in enumerate(pair):
                        t, j = y_row[c]
                        nc.tensor.matmul(
                            st['ps'],
                            t[:, j, sb4 * 128:(sb4 + 1) * 128],
                            WO[c], start=(first and k == 0),
                            stop=(last and k == len(pair) - 1))
                return go

            def op_out(i, sb4, o_t):
                sb = i * (PANEL // 128) + sb4
                # two DMAs land on separate queues -> 2x drain bw
                r = slice(sb * 128, (sb + 1) * 128)
                nc.sync.dma_start(out=out[r, 0:OUTW // 2],
                                  in_=o_t[:, 0:OUTW // 2])
                nc.sync.dma_start(out=out[r, OUTW // 2:OUTW],
                                  in_=o_t[:, OUTW // 2:OUTW])

            def outproj_micros(i, subs=None):
                micros = []

                def epi(sb4, st):
                    def go():
                        o_t = misc.tile([128, OUTW], F16, tag="o_t", bufs=3,
                                        name="o_t")
                        nc.vector.tensor_add(o_t, st['ps'], bo_bc)
                        op_out(i, sb4, o_t)
                    return go

                pairs = ((0, 1), (2, 3), (4, 5), (6, 7))
                for sb4 in (subs if subs is not None
                            else range(PANEL // 128)):
                    st = {}
                    for k, pair in enumerate(pairs):
                        micros.append(op_mms(i, sb4, st, pair, k == 0,
                                             k == len(pairs) - 1))
                    micros.append(epi(sb4, st))
                return micros

            def outproj_final(i):
                """Two-phase out-proj for the last panel.  Phase 1 (6 of 8
                chunks, gathered by the heads-0-3 and heads-4-5
                collectives) runs right after the last attention slot,
                filling the PE while the last head norm + final collective
                + readback chain drains.  Phase 2 (chunks 3,7 + partial
                add) is all that waits on the final 128KB gather."""
                ph1, ph2 = [], []
                parts = {}

                def p1_epi(sb4, st):
                    def go():
                        part = misc.tile([128, OUTW], F16, tag="op_part",
                                         bufs=4, name="op_part")
                        parts[sb4] = part
                        nc.vector.tensor_add(part, st['ps'], bo_bc)
                    return go

                def p2_epi(sb4, st):
                    def go():
                        o_t = misc.tile([128, OUTW], F16, tag="o_t", bufs=3,
                                        name="o_t")
                        nc.vector.tensor_add(o_t, st['ps'], parts[sb4])
                        op_out(i, sb4, o_t)
                    return go

                P1 = ((0, 1), (4, 5), (2, 6))
                for sb4 in range(PANEL // 128):
                    st = {}
                    for k, pair in enumerate(P1):
                        ph1.append(op_mms(i, sb4, st, pair, k == 0,
                                          k == len(P1) - 1))
                    ph1.append(p1_epi(sb4, st))
                for sb4 in range(PANEL // 128):
                    st = {}
                    ph2.append(op_mms(i, sb4, st, (3, 7), True, True))
                    ph2.append(p2_epi(sb4, st))
                return ph1, ph2

            def interleave(slots, fillers):
                n, m = len(slots), len(fillers)
                fi = 0
                for si, s in enumerate(slots):
                    s()
                    while fi < m and fi * n <= (si + 1) * m:
                        fillers[fi]()
                        fi += 1
                while fi < m:
                    fillers[fi]()
                    fi += 1

            # ---------- schedule ----------
            def run_round(pa, fillers, final=False):
                if final:
                    # finer collective splits so the last gather is only
                    # heads 6-7 (128KB): chunks 3,7 are all that's left of
                    # outproj(pa) after the round
                    segs = [(0, 4, y_allA),
                            (4, 6, y_allF1),
                            (6, 8, y_allF2)]
                else:
                    segs = [(0, 8, y_allN[pa])]
                slots = [[s for f in range(a // 2, b // 2)
                          for s in pair_slots(pa, f)] for a, b, _ in segs]
                total = sum(len(g) for g in slots)
                done = 0
                k0 = 0
                for (a, b, y_out), sl in zip(segs, slots):
                    done += len(sl)
                    k1 = len(fillers) * done // total
                    interleave(sl, fillers[k0:k1])
                    emit_cc_heads(pa, a, b, y_out)
                    k0 = k1
                QTs.pop(pa, None)

            # round 0: proj(0) straight
            for mo in proj_micros(0, xT0):
                mo()
            # rounds 1..NP-1: attn(p-1) x [proj(p) + outproj(p-2) +
            # early S+exp of attn(p)'s first off-diagonal key groups]
            # Cross-round pre-S was measured performance-neutral (the
            # final round is PE/ACT balanced, not ACT-bound as modeled):
            # machinery kept dormant
            PRE = {}
            for p in range(1, NP):
                fillers = proj_micros(p, emit_xT(p))
                if p >= 2:
                    fillers += outproj_micros(p - 2)
                # pre-S last: they need QT(p) written by this round's proj
                nbufs = 2 * FG * len(PRE.get(p, ()))
                for f in range(FG):
                    for j in PRE.get(p, ()):
                        fillers.append(pre_spair(p, f, j, nbufs))
                run_round(p - 1, fillers)
            # final round: attn(NP-1) is ACT(Exp)-bound, so PE filler work
            # is free to move past it.  Keep half of outproj(NP-2) as
            # in-round fillers; the other half plus phase 1 of
            # outproj(NP-1) run after the last attention slot, covering
            # the last head's norm + final collective + readback chain.
            # Phase 2 (8 matmuls) is all that waits on the final gather.
            ph1, ph2 = outproj_final(NP - 1)
            run_round(NP - 1, outproj_micros(NP - 2), final=True)
            for mo in ph1:
                mo()
            for mo in ph2:
                mo()

    nc.compile()
    return nc


def shard_inputs(x, mask, Wqkv, bqkv, Wo, bo, sl=SL, ne=NE, nh=NH):
    """Host-side sharding: returns in_maps for the 8 cores."""
    H = nh // 2
    F = H * HD
    scale = 1.0 / np.sqrt(HD)
    bv_full = bqkv[2 * ne:3 * ne]
    bo_eff = bo + bv_full @ Wo  # V bias folded through the out projection
    in_maps = []
    for c in range(N_CORES):
        b, g = c // 2, c % 2
        qc = slice(g * F, (g + 1) * F)
        kc = slice(ne + g * F, ne + (g + 1) * F)
        vc = slice(2 * ne + g * F, 2 * ne + (g + 1) * F)
        oc = slice(g * (ne // 2), (g + 1) * (ne // 2))
        in_maps.append({
            "xt": np.ascontiguousarray(x[b].T).astype(np.float16),
            "wq": (np.ascontiguousarray(Wqkv[:, qc]) * scale).astype(np.float16),
            "wk": np.ascontiguousarray(Wqkv[:, kc]).astype(np.float16),
            "wv": np.ascontiguousarray(Wqkv[:, vc]).astype(np.float16),
            "bq": (np.ascontiguousarray(bqkv[qc]) * scale).astype(np.float32),
            "wo": np.ascontiguousarray(Wo[:, oc]).astype(np.float16),
            "bo": np.ascontiguousarray(bo_eff[oc]).astype(np.float32),
        })
    return in_maps


def unshard_output(results, sl=SL, ne=NE):
    out = np.empty((BS, sl, ne), dtype=np.float32)
    half = ne // 2
    for c in range(N_CORES):
        b, g = c // 2, c % 2
        out[b, :, g * half:(g + 1) * half] = results[c]["out"].astype(
            np.float32)
    return out


_NC_CACHE = {}


def kernel(x, mask, Wqkv, bqkv, Wo, bo):
    x = np.asarray(x, dtype=np.float32)
    Wqkv = np.asarray(Wqkv, dtype=np.float32)
    bqkv = np.asarray(bqkv, dtype=np.float32)
    Wo = np.asarray(Wo, dtype=np.float32)
    bo = np.asarray(bo, dtype=np.float32)
    if "nc" not in _NC_CACHE:
        _NC_CACHE["nc"] = build_nc()
    nc = _NC_CACHE["nc"]
    in_maps = shard_inputs(x, mask, Wqkv, bqkv, Wo, bo)
    res = run_bass_kernel_spmd(nc, in_maps, list(range(N_CORES)))
    return unshard_output(res.results)

